# revision 21
# baseline (speedup 1.0000x reference)
"""AttentionBlock (GroupNorm32 + 8-head global self-attention + proj + residual)
on 8 TRN2 NeuronCores, data-parallel over batch (B=8 -> 1 image per core).

v3: ACT-exp is the critical resource (~73us of exp at 1 elem/lane/cycle).
The kernel is a software pipeline that keeps ScalarE busy on exp while PE
fills its slack with QKV / V / PV / proj work, staying HAM-warm:

  - x arrives in 4 channel-tile DMAs spread over 4 queues; GroupNorm stats
    run per-tile as x lands, with one batched [8,4]-shaped tail chain.
  - PE warmup matmuls on a zeroed tile un-throttle HAM before real work.
  - S^T(pair) matmuls 2-way row-packed (even head rows 0-63, odd 64-127).
  - exp(scale*S^T) on ACT from PSUM [128,1024] tiles, double-buffered.
  - PV per (head, n-half) accumulates [66,512]; row 64 = ones gives the
    softmax denominators. Normalize: denom rows reshaped onto 128
    partitions by SBUF->SBUF DMA, cheap [128,8] DVE reciprocal, DMA back
    to K-rows {0,32}, K=33 matmul broadcast, one DVE multiply.
  - proj streams per [128,512] chunk with residual, DMA'd out as produced.
"""
import numpy as np

C = 512
NH = 8
D = 64
N = 1024
GROUPS = 32
GS = C // GROUPS  # 16 channels per group
EPS = 1e-5
B = 8
CT = C // 128      # 4 channel tiles (= head pairs)
MT = N // 128      # 8 m-tiles
NHF = 2            # n halves of 512

TRACE = False     # test.py flips this for profiling runs

_cache = {}


def _build():
    import concourse.bacc as bacc
    import concourse.tile as tile
    import concourse.mybir as mybir

    F32 = mybir.dt.float32
    F32R = mybir.dt.float32r
    BF16 = mybir.dt.bfloat16
    AF = mybir.ActivationFunctionType
    ALU = mybir.AluOpType
    nc = bacc.Bacc("TRN2", target_bir_lowering=False, debug=False,
                   enable_asserts=False, num_devices=1)

    x_d = nc.dram_tensor("x", [C, N], F32, kind="ExternalInput").ap()
    qkv_wT_d = nc.dram_tensor("qkv_wT", [C, 3 * C], BF16, kind="ExternalInput").ap()
    proj_wT_d = nc.dram_tensor("proj_wT", [C, C], BF16, kind="ExternalInput").ap()
    # packed per-channel params: cols 0-3 gn_w, 4-7 gn_b, 8-15 qk_bias, 16-19 proj_be
    pp_d = nc.dram_tensor("pp", [128, 20], F32, kind="ExternalInput").ap()
    sel_d = nc.dram_tensor("sel", [128, 8], F32R, kind="ExternalInput").ap()
    expd_d = nc.dram_tensor("expd", [8, 128], F32R, kind="ExternalInput").ap()
    # denominator-broadcast selector: row 0 -> cols 0-63, row 32 -> cols 64-127
    sel2_d = nc.dram_tensor("sel2", [33, 128], F32R, kind="ExternalInput").ap()
    out_d = nc.dram_tensor("out", [C, N], F32, kind="ExternalOutput").ap()

    scale = float(D) ** -0.5

    with tile.TileContext(nc) as tc:
        with tc.tile_pool(name="const", bufs=1) as const, \
             tc.tile_pool(name="big", bufs=1) as big, \
             tc.tile_pool(name="pT_pool", bufs=4) as pT_pool, \
             tc.tile_pool(name="small", bufs=2) as small, \
             tc.tile_pool(name="rsp", bufs=2) as rsp, \
             tc.tile_pool(name="outp", bufs=2) as outp, \
             tc.tile_pool(name="sp", bufs=2, space="PSUM") as sp, \
             tc.tile_pool(name="pm", bufs=4, space="PSUM") as pm:

            # ---- DMA in: packed params (tiny) + x spread over 4 queues
            pp = const.tile([128, 20], F32)
            sel = const.tile([128, 8], F32R)
            expd = const.tile([8, 128], F32R)
            sel2 = const.tile([33, 128], F32R)
            rs2 = const.tile([33, 512], F32R)
            z33 = const.tile([33, 512], F32)
            eps_t = const.tile([8, 1], F32)
            nc.vector.memset(z33, 0.0)
            nc.vector.tensor_copy(out=rs2, in_=z33)
            nc.vector.memset(eps_t, EPS)
            nc.sync.dma_start(out=pp, in_=pp_d)
            nc.scalar.dma_start(out=sel, in_=sel_d)
            nc.scalar.dma_start(out=expd, in_=expd_d)
            nc.scalar.dma_start(out=sel2, in_=sel2_d)
            gn_w = pp[:, 0:4]
            gn_b = pp[:, 4:8]
            qk_bias = pp[:, 8:16].rearrange("p (t o) -> p t o", o=1)
            proj_be = pp[:, 16:20].rearrange("p (t o) -> p t o", o=1)

            x_sb = big.tile([128, CT, N], F32)
            x_eng = [nc.sync, nc.scalar, nc.gpsimd, nc.sync]
            for ci in range(CT):
                x_eng[ci].dma_start(out=x_sb[:, ci, :],
                                    in_=x_d[128 * ci:128 * (ci + 1), :])
            # weights on the gpsimd issue queue (parallel with the above)
            qkv_wT = const.tile([128, CT, 3 * C], BF16)
            proj_wT = const.tile([128, CT, C], BF16)
            nc.gpsimd.dma_start(
                out=qkv_wT[:, :, 0:2 * C],
                in_=qkv_wT_d[:, 0:2 * C].rearrange("(t p) o -> p t o", p=128))
            nc.gpsimd.dma_start(
                out=qkv_wT[:, :, 2 * C:3 * C],
                in_=qkv_wT_d[:, 2 * C:3 * C].rearrange("(t p) o -> p t o", p=128))
            nc.gpsimd.dma_start(out=proj_wT,
                                in_=proj_wT_d.rearrange("(t p) o -> p t o", p=128))

            # ---- PE warmup: un-throttle HAM on junk matmuls over zeros ----
            wu_ps = pm.tile([128, 512], F32, tag="pm", name="wu")
            for w in range(8):
                nc.tensor.matmul(wu_ps[:], rs2[:, 0:128], rs2[:],
                                 start=True, stop=True)

            # ---- GroupNorm: per-tile stats, one batched tail chain ----
            hn = big.tile([128, CT, N], BF16)
            gall = small.tile([8, CT, 2], F32, tag="gall", bufs=1)
            for ci in range(CT):
                bstats = small.tile([128, 2, 6], F32, tag="bstats")
                xv = x_sb[:, ci, :].rearrange("p (s n) -> p s n", s=2)
                for s in range(2):
                    nc.vector.bn_stats(out=bstats[:, s, :], in_=xv[:, s, :])
                mv = small.tile([128, 2], F32, tag="mv")
                nc.vector.bn_aggr(out=mv, in_=bstats)
                # stat_rhs = [mean_c, var_c + mean_c^2]
                stat_rhs = small.tile([128, 2], F32R, tag="statr")
                nc.vector.tensor_copy(out=stat_rhs[:, 0:1], in_=mv[:, 0:1])
                nc.vector.tensor_tensor(out=stat_rhs[:, 1:2], in0=mv[:, 0:1],
                                        in1=mv[:, 0:1], op=ALU.mult)
                nc.vector.tensor_tensor(out=stat_rhs[:, 1:2], in0=stat_rhs[:, 1:2],
                                        in1=mv[:, 1:2], op=ALU.add)
                grp_ps = pm.tile([8, 2], F32, tag="pm", name=f"grp_ps{ci}")
                nc.tensor.matmul(grp_ps[:], sel, stat_rhs, start=True, stop=True)
                nc.vector.tensor_scalar(out=gall[:, ci, :], in0=grp_ps[:],
                                        scalar1=1.0 / GS, scalar2=None,
                                        op0=ALU.mult)
            # batched group chain: var = E[x^2] - mean^2 ; rstd = 1/sqrt(var+eps)
            gmean = gall[:, :, 0:1]
            gvar = gall[:, :, 1:2]
            gm2 = small.tile([8, CT, 1], F32, tag="gm2", bufs=1)
            nc.vector.tensor_tensor(out=gm2, in0=gmean, in1=gmean, op=ALU.mult)
            nc.vector.tensor_tensor(out=gvar, in0=gvar, in1=gm2, op=ALU.subtract)
            nc.scalar.activation(out=gvar, in_=gvar, func=AF.Sqrt,
                                 bias=eps_t, scale=1.0)
            nc.vector.reciprocal(out=gvar, in_=gvar)
            g2 = small.tile([8, CT, 2], F32R, tag="g2", bufs=1)
            nc.vector.tensor_copy(out=g2, in_=gall)
            absb = small.tile([128, CT, 2], F32, tag="absb", bufs=1)
            for ci in range(CT):
                ab_ps = pm.tile([128, 2], F32, tag="pm", name=f"ab_ps{ci}")
                nc.tensor.matmul(ab_ps[:], expd, g2[:, ci, :], start=True, stop=True)
                nc.vector.tensor_copy(out=absb[:, ci, :], in_=ab_ps[:])
            A_all = small.tile([128, CT], F32, tag="A", bufs=1)
            B_all = small.tile([128, CT], F32, tag="Bb", bufs=1)
            nc.vector.tensor_tensor(out=A_all, in0=absb[:, :, 1], in1=gn_w,
                                    op=ALU.mult)
            nc.vector.tensor_tensor(out=B_all, in0=absb[:, :, 0], in1=A_all,
                                    op=ALU.mult)
            nc.vector.tensor_tensor(out=B_all, in0=gn_b, in1=B_all,
                                    op=ALU.subtract)
            for ci in range(CT):
                nc.vector.tensor_scalar(out=hn[:, ci, :], in0=x_sb[:, ci, :],
                                        scalar1=A_all[:, ci:ci + 1],
                                        scalar2=B_all[:, ci:ci + 1],
                                        op0=ALU.mult, op1=ALU.add)

            # ---- building blocks ----
            q_sb = big.tile([128, CT, N], BF16)
            k_sb = big.tile([128, CT, N], BF16)
            vT = big.tile([128, MT, NH, D + 2], BF16)
            oT = big.tile([128, CT, N], BF16)
            nc.vector.memset(vT[:, :, :, D:D + 1], 1.0)
            nc.vector.memset(vT[:, :, :, D + 1:D + 2], 0.0)

            def qkv_pair(t):
                for which in range(2):  # 0=q, 1=k
                    dest = q_sb if which == 0 else k_sb
                    bt = which * CT + t
                    for nh in range(NHF):
                        ps = pm.tile([128, 512], F32, tag="pm",
                                     name=f"qk{t}_{which}_{nh}")
                        for kt in range(CT):
                            nc.tensor.matmul(
                                ps[:],
                                qkv_wT[:, kt, 128 * bt:128 * (bt + 1)],
                                hn[:, kt, 512 * nh:512 * (nh + 1)],
                                start=(kt == 0), stop=(kt == CT - 1))
                        nc.vector.tensor_scalar(
                            out=dest[:, t, 512 * nh:512 * (nh + 1)], in0=ps[:],
                            scalar1=qk_bias[:, bt, :], scalar2=None, op0=ALU.add)

            def v_tile(mt):
                ps = pm.tile([128, 512], F32, tag="pm", name=f"v{mt}")
                for kt in range(CT):
                    nc.tensor.matmul(ps[:], hn[:, kt, 128 * mt:128 * (mt + 1)],
                                     qkv_wT[:, kt, 2 * C:3 * C],
                                     start=(kt == 0), stop=(kt == CT - 1))
                nc.vector.tensor_copy(
                    out=vT[:, mt, :, 0:D],
                    in_=ps[:].rearrange("p (h d) -> p h d", h=NH))

            pT_of = {}

            def s_exp(t):
                # S^T + exp for pair t; even/odd head row-packed concurrently.
                pts = [pT_pool.tile([128, MT, N], BF16, tag="pT", name=f"pT_{t}_{hh}")
                       for hh in range(2)]
                pT_of[t] = pts
                for mt in range(MT):
                    for hh in range(2):
                        qp = hh * 64
                        ps_s = sp.tile([128, N], F32, tag="sT",
                                       name=f"s_{t}_{mt}_{hh}")
                        for nh in range(NHF):
                            nc.tensor.matmul(
                                ps_s[:, 512 * nh:512 * (nh + 1)],
                                k_sb[qp:qp + 64, t, 128 * mt:128 * (mt + 1)],
                                q_sb[qp:qp + 64, t, 512 * nh:512 * (nh + 1)],
                                start=True, stop=True)
                        nc.scalar.activation(out=pts[hh][:, mt, :], in_=ps_s[:],
                                             func=AF.Exp, scale=scale)

            def pv_nh(t, nh, release):
                # PV for both heads of pair t, n-half nh; then normalize.
                pts = pT_of[t]
                pso = []
                for hh in range(2):
                    ps_o = pm.tile([D + 2, 512], F32, tag="pm",
                                   name=f"pso_{t}_{nh}_{hh}")
                    pso.append(ps_o)
                for mt in range(MT):
                    for hh in range(2):
                        nc.tensor.matmul(pso[hh][:],
                                         vT[:, mt, 2 * t + hh, :],
                                         pts[hh][:, mt, 512 * nh:512 * (nh + 1)],
                                         start=(mt == 0), stop=(mt == MT - 1))
                if release:
                    pT_of.pop(t)
                ns = slice(512 * nh, 512 * (nh + 1))
                # drain o and the denominator rows
                rsb = rsp.tile([1, 2, 512], F32, tag="rsb", name=f"rsb_{t}_{nh}")
                for hh in range(2):
                    qp = hh * 64
                    nc.vector.tensor_copy(out=oT[qp:qp + 64, t, ns],
                                          in_=pso[hh][0:D, :])
                    nc.vector.tensor_copy(out=rsb[0:1, hh, :],
                                          in_=pso[hh][D:D + 1, :])
                # reshape [2,512] denom rows onto 128 partitions, recip, back
                rT = rsp.tile([128, 2, 4], F32, tag="rT", name=f"rT_{t}_{nh}")
                for hh in range(2):
                    nc.sync.dma_start(
                        out=rT[:, hh, :],
                        in_=rsb[0:1, hh, :].rearrange("o (p j) -> o p j", p=128))
                nc.vector.reciprocal(out=rT, in_=rT)
                for hh in range(2):
                    nc.gpsimd.dma_start(
                        out=rs2[32 * hh:32 * hh + 1, :].rearrange(
                            "o (p j) -> o p j", p=128),
                        in_=rT[:, hh, :])
                bc_ps = pm.tile([128, 512], F32, tag="pm", name=f"bc_{t}_{nh}")
                nc.tensor.matmul(bc_ps[:], sel2, rs2, start=True, stop=True)
                nc.vector.tensor_tensor(out=oT[:, t, ns], in0=oT[:, t, ns],
                                        in1=bc_ps[:], op=ALU.mult)

            def proj_nh(nh):
                ns = slice(512 * nh, 512 * (nh + 1))
                for ot in range(CT):
                    ps = pm.tile([128, 512], F32, tag="pm", name=f"pr{ot}_{nh}")
                    for kt in range(CT):
                        nc.tensor.matmul(ps[:],
                                         proj_wT[:, kt, 128 * ot:128 * (ot + 1)],
                                         oT[:, kt, ns],
                                         start=(kt == 0), stop=(kt == CT - 1))
                    oc = outp.tile([128, 512], F32, tag="oc", name=f"oc{ot}_{nh}")
                    nc.vector.scalar_tensor_tensor(
                        out=oc, in0=ps[:], scalar=proj_be[:, ot, :],
                        in1=x_sb[:, ot, ns], op0=ALU.add, op1=ALU.add)
                    nc.gpsimd.dma_start(out=out_d[128 * ot:128 * (ot + 1), ns],
                                        in_=oc)

            # ---- emission order = scheduler priority ----
            qkv_pair(0)
            s_exp(0)
            for mt in range(MT):
                v_tile(mt)
            qkv_pair(1)
            pv_nh(0, 0, release=False)
            s_exp(1)
            pv_nh(0, 1, release=True)
            qkv_pair(2)
            pv_nh(1, 0, release=False)
            s_exp(2)
            pv_nh(1, 1, release=True)
            qkv_pair(3)
            pv_nh(2, 0, release=False)
            s_exp(3)
            pv_nh(2, 1, release=True)
            pv_nh(3, 0, release=False)
            pv_nh(3, 1, release=True)
            proj_nh(0)
            proj_nh(1)

    nc.compile()
    return nc


def _host_prep(x, gn_w, gn_b, qkv_w, qkv_b, proj_w, proj_b):
    xf = np.ascontiguousarray(x.reshape(B, C, N), dtype=np.float32)
    import ml_dtypes
    qkv_wT = np.ascontiguousarray(qkv_w.T).astype(ml_dtypes.bfloat16)
    proj_wT = np.ascontiguousarray(proj_w.T).astype(ml_dtypes.bfloat16)
    proj_be = (proj_b + proj_w @ qkv_b[2 * C:]).astype(np.float32)
    qk_bias = np.asarray(qkv_b[:2 * C], dtype=np.float32)
    # packed per-channel params [128, 20]: col-major by channel tile
    pp = np.zeros((128, 20), np.float32)
    pp[:, 0:4] = np.asarray(gn_w, np.float32).reshape(4, 128).T
    pp[:, 4:8] = np.asarray(gn_b, np.float32).reshape(4, 128).T
    pp[:, 8:16] = qk_bias.reshape(8, 128).T
    pp[:, 16:20] = proj_be.reshape(4, 128).T
    # per-tile group selector: channel p (within tile) -> group p//16 (of 8)
    sel = (np.arange(128)[:, None] // GS == np.arange(8)[None, :]).astype(np.float32)
    expd = np.ascontiguousarray(sel.T)
    sel2 = np.zeros((33, 128), np.float32)
    sel2[0, 0:64] = 1.0
    sel2[32, 64:128] = 1.0
    shared = {
        "qkv_wT": qkv_wT, "proj_wT": proj_wT, "pp": pp,
        "sel": sel, "expd": expd, "sel2": sel2,
    }
    return [{**shared, "x": np.ascontiguousarray(xf[i])} for i in range(B)]


def kernel(x, gn_w, gn_b, qkv_w, qkv_b, proj_w, proj_b):
    from concourse import bass_utils
    in_maps = _host_prep(np.asarray(x), np.asarray(gn_w), np.asarray(gn_b),
                         np.asarray(qkv_w), np.asarray(qkv_b),
                         np.asarray(proj_w), np.asarray(proj_b))
    if "nc" not in _cache:
        _cache["nc"] = _build()
    res = bass_utils.run_bass_kernel_spmd(_cache["nc"], in_maps,
                                          core_ids=list(range(B)), trace=TRACE)
    _cache["last_result"] = res
    out = np.stack([res.results[i]["out"] for i in range(B)])
    return out.reshape(B, C, 32, 32).astype(np.float32)
